# revision 1
# baseline (speedup 1.0000x reference)
"""Spatial self-attention scores kernel for Trainium2 (8 NeuronCores).

Computes, per batch b:
    qk = W @ x_b          # [256, 4096] = [256,256] @ [256,4096]
    q, k = qk[:128], qk[128:]
    sim = (q.T @ k) * 128**-0.5
    out_b = softmax(sim, axis=-1)        # [4096, 4096]
Output: [8, 1, 4096, 4096] float32.

Sharding: data-parallel over batch, one batch image per NeuronCore.

Per-core pipeline (all phases overlap under the Tile scheduler):
  - x DMA'd in as fp16 (SWDGE cast); W transposed on PE via identity.
  - fp16 projection matmuls -> q,k in SBUF as [d=128, s=4096] float32r,
    interleaved with the first attention groups so the in-order PE
    reaches the first output as early as possible.
  - per 128-query row-tile: 8 fp32r matmuls (K=128, N=512) into 4-bank
    PSUM tiles; one ScalarE ACTIVATE per 2048 columns computes
    exp(SCALE*sim) with a fused row-sum (accum_out); DVE combines the
    partial sums, takes the reciprocal, and scales the row.
  - output rows leave in 4 MB DMAs (two row-tiles per transfer; the
    first group ships per normalized half-row).
"""

import numpy as np
from contextlib import ExitStack

import concourse.bass as bass
import concourse.tile as tile
from concourse import bacc, mybir
from concourse.bass_utils import run_bass_kernel_spmd
from concourse.masks import make_identity

B = 8
C = 256
HW = 4096
D = 128
SCALE = D ** -0.5
N_CORES = 8

BANK = 512             # PSUM bank width (fp32) = one matmul free-dim
ACT_CHUNK = 2048       # one ScalarE activation spans 4 banks
N_ACT = HW // ACT_CHUNK          # 2
GRP = 2                # row-tiles per output DMA (2 -> 4 MB transfers)
N_GRP = HW // (128 * GRP)        # 16
OUT_BUFS = 4
X_CHUNK = 1024         # x input DMA granularity (overlaps with projection)

F32 = mybir.dt.float32
# float32r streams through the PE at 2 cycles/row (vs 4 for float32)
# with near-fp32 precision (measured ~3e-4 scale-relative on this
# kernel). The BIR verifier requires fp32r operands to be *produced*
# as fp32r, so operand tiles carry this dtype and their producers
# (SWDGE cast DMA / DVE copies) write it directly.
MM_DT = mybir.dt.float32r
# The projection runs in fp16: halves x's SBUF footprint (freeing room
# for a 4th output buffer) and the input DMA bytes, and streams the PE
# at 1 cycle/row. q/k and the big attention matmuls stay float32r.
# fp16's 10-bit mantissa keeps the extra logit noise ~4x below bf16's
# (values here are well within fp16 range).
PROJ_DT = mybir.dt.float16


def _emit(ctx: ExitStack, tc: tile.TileContext, out_ap, x_ap, w_ap):
    nc = tc.nc

    const = ctx.enter_context(tc.tile_pool(name="const", bufs=1))
    data = ctx.enter_context(tc.tile_pool(name="data", bufs=1))
    psum = ctx.enter_context(tc.tile_pool(name="psum", bufs=2, space="PSUM"))
    small = ctx.enter_context(tc.tile_pool(name="small", bufs=4))

    # ---- PE warm-up: throwaway matmuls while x is loading. The PE
    # clock gate (HAM) only releases to 2.4 GHz after ~3.4 us of
    # sustained activity; warming during the input DMA makes the
    # projection and the first attention row-tiles run at full rate.
    warm_f32 = const.tile([128, BANK], F32)
    nc.vector.memset(warm_f32, 0.0)
    warm = const.tile([128, BANK], MM_DT)
    nc.vector.tensor_copy(out=warm, in_=warm_f32)
    wps = psum.tile([128, ACT_CHUNK], F32, tag="ps")
    for _ in range(4):
        nc.tensor.matmul(
            wps[:, 0:BANK], warm[:, 0:128], warm, start=True, stop=True
        )

    ident = const.tile([128, 128], F32)
    make_identity(nc, ident)

    # ---- W [256, 256] -> SBUF as [p, o_tile, c]
    w_sb = const.tile([128, 2, C], F32)
    nc.sync.dma_start(out=w_sb, in_=w_ap.rearrange("(t p) c -> p t c", p=128))
    # pull the exp table load off the first real activation
    tbl = small.tile([128, 1], F32, tag="tbl")
    nc.scalar.activation(
        out=tbl, in_=warm_f32[:, 0:1], func=mybir.ActivationFunctionType.Exp
    )

    # ---- transpose W on PE -> wt_sb[c_sub, c_tile, o] (contraction c on partitions)
    wt_sb = const.tile([128, 2, 2 * D], PROJ_DT)
    for t in range(2):          # output-channel tile (q half / k half)
        for ct in range(2):     # input-channel tile
            ps = psum.tile([128, ACT_CHUNK], F32, tag="ps")
            nc.tensor.transpose(
                ps[:, 0:128], w_sb[:, t, ct * 128:(ct + 1) * 128], ident
            )
            nc.vector.tensor_copy(
                out=wt_sb[:, ct, t * 128:(t + 1) * 128], in_=ps[:, 0:128]
            )

    q_sb = data.tile([128, HW], MM_DT)
    k_sb = data.tile([128, HW], MM_DT)

    def proj_chunk(t, dst, a, x_half, banks=None):
        """Project output-channel half t for column chunk a; x_half is
        [128, chunk, c_tile, X_CHUNK] holding x columns
        [a*ACT_CHUNK, (a+1)*ACT_CHUNK). banks selects a subset of the
        four 512-wide banks."""
        ps = psum.tile([128, ACT_CHUNK], F32, tag="ps")
        for jj in banks if banks is not None else range(ACT_CHUNK // BANK):
            sl = slice(a * ACT_CHUNK + jj * BANK,
                       a * ACT_CHUNK + (jj + 1) * BANK)
            lo = slice(jj * BANK, (jj + 1) * BANK)
            ch = (jj * BANK) // X_CHUNK
            off = (jj * BANK) % X_CHUNK
            nc.tensor.matmul(
                ps[:, lo], wt_sb[:, 0, t * 128:(t + 1) * 128],
                x_half[:, ch, 0, off:off + BANK], start=True, stop=False,
            )
            nc.tensor.matmul(
                ps[:, lo], wt_sb[:, 1, t * 128:(t + 1) * 128],
                x_half[:, ch, 1, off:off + BANK], start=False, stop=True,
            )
            nc.vector.tensor_copy(out=dst[:, sl], in_=ps[:, lo])

    outp = None
    out_view = out_ap.rearrange("(g t p) m -> g p t m", t=GRP, p=128)

    def sim_chunk(lhs, out_row, lo_col, n_col, accum):
        """n_col-wide slice of one attention row: matmuls + fused exp."""
        ps = psum.tile([128, ACT_CHUNK], F32, tag="ps")
        for jj in range(n_col // BANK):
            sl = slice(lo_col + jj * BANK, lo_col + (jj + 1) * BANK)
            nc.tensor.matmul(
                ps[:, jj * BANK:(jj + 1) * BANK], lhs, k_sb[:, sl],
                start=True, stop=True,
            )
        nc.scalar.activation(
            out=out_row[:, lo_col:lo_col + n_col],
            in_=ps[:, 0:n_col],
            func=mybir.ActivationFunctionType.Exp,
            scale=SCALE,
            accum_out=accum,
        )

    def emit_group(g, split_dma=False, fine=False):
        out_grp = outp.tile([128, GRP, HW], F32, tag="out")
        for t in range(GRP):
            i = g * GRP + t
            lhs = q_sb[:, i * 128:(i + 1) * 128]
            if fine and t == 0:
                # Fast path for the very first attention row: its second
                # column chunk runs as two 1024-wide pieces interleaved
                # with k chunk 1's projection banks, so the last exp (and
                # with it the first output byte) fires ~2 us earlier.
                sums = small.tile([128, 3], F32, tag="sums")
                sim_chunk(lhs, out_grp[:, t], 0, ACT_CHUNK, sums[:, 0:1])
                proj_chunk(1, k_sb, 1, x1_sb, banks=(0, 1))
                sim_chunk(lhs, out_grp[:, t], ACT_CHUNK, 1024, sums[:, 1:2])
                proj_chunk(1, k_sb, 1, x1_sb, banks=(2, 3))
                sim_chunk(lhs, out_grp[:, t], ACT_CHUNK + 1024, 1024,
                          sums[:, 2:3])
            else:
                sums = small.tile([128, N_ACT], F32, tag="sums")
                for a in range(N_ACT):
                    sim_chunk(lhs, out_grp[:, t], a * ACT_CHUNK, ACT_CHUNK,
                              sums[:, a:a + 1])
            rsum = small.tile([128, 1], F32, tag="rsum")
            nc.vector.tensor_reduce(
                out=rsum, in_=sums, axis=mybir.AxisListType.X,
                op=mybir.AluOpType.add,
            )
            recip = small.tile([128, 1], F32, tag="recip")
            nc.vector.reciprocal(out=recip, in_=rsum)
            if split_dma:
                # normalize and ship each half-row as soon as it is
                # scaled (1 MB transfers) so the first outputs leave at
                # the earliest possible moment
                i = g * GRP + t
                for a in range(N_ACT):
                    sl = slice(a * ACT_CHUNK, (a + 1) * ACT_CHUNK)
                    nc.vector.tensor_scalar_mul(
                        out=out_grp[:, t, sl], in0=out_grp[:, t, sl],
                        scalar1=recip,
                    )
                    nc.sync.dma_start(
                        out=out_ap[i * 128:(i + 1) * 128, sl],
                        in_=out_grp[:, t, sl],
                    )
            else:
                nc.vector.tensor_scalar_mul(
                    out=out_grp[:, t, :], in0=out_grp[:, t, :], scalar1=recip
                )
        if not split_dma:
            nc.sync.dma_start(out=out_view[g], in_=out_grp)

    # x loaded with an SWDGE cast straight to fp16. Each chunk DMA
    # writes a contiguous [chunk, c_tile, cols] block so projection
    # banks depend only on their own chunk's transfer.
    x_view = x_ap.rearrange("(t p) s -> p t s", p=128)
    x0_sb = data.tile([128, ACT_CHUNK // X_CHUNK, 2, X_CHUNK], PROJ_DT)
    x1_sb = data.tile([128, ACT_CHUNK // X_CHUNK, 2, X_CHUNK], PROJ_DT)
    for half, dst_x in ((0, x0_sb), (1, x1_sb)):
        for c in range(ACT_CHUNK // X_CHUNK):
            src = slice(half * ACT_CHUNK + c * X_CHUNK,
                        half * ACT_CHUNK + (c + 1) * X_CHUNK)
            nc.gpsimd.dma_start(out=dst_x[:, c], in_=x_view[:, :, src])

    # ---- projection, interleaved with the attention groups so the
    # in-order PE reaches the first output DMA as early as possible:
    #   k chunk 0, q bank 0 (rows 0-511) -> group 0 can start; k chunk 1
    #   is emitted between group 0's first and second column chunks; the
    #   rest of q follows behind the early groups.
    proj_chunk(1, k_sb, 0, x0_sb)               # k cols 0:2048
    proj_chunk(0, q_sb, 0, x0_sb, banks=(0,))   # q rows 0:512

    outp = ctx.enter_context(tc.tile_pool(name="outp", bufs=OUT_BUFS))
    emit_group(0, split_dma=True, fine=True)
    # remaining q projections trickle in one 512-wide bank at a time,
    # each just ahead of the first group that reads it, so the PE insert
    # never exceeds ~1 us between groups
    emit_group(1)
    proj_chunk(0, q_sb, 0, x0_sb, banks=(1,))   # rows  512:1024 (grps 2-3)
    emit_group(2)
    proj_chunk(0, q_sb, 0, x0_sb, banks=(2,))   # rows 1024:1536 (grps 4-5)
    emit_group(3)
    proj_chunk(0, q_sb, 0, x0_sb, banks=(3,))   # rows 1536:2048 (grps 6-7)
    for g in range(4, N_GRP // 2):
        emit_group(g)
        # q chunk 1 (row-tiles 16-31), one bank ahead of groups 8-11
        proj_chunk(0, q_sb, 1, x1_sb, banks=(g - 4,))
    for g in range(N_GRP // 2, N_GRP):
        emit_group(g)


_built = None


def _get_nc():
    global _built
    if _built is None:
        nc = bacc.Bacc("TRN2", target_bir_lowering=False, debug=False)
        x = nc.dram_tensor("x", [C, HW], F32, kind="ExternalInput").ap()
        w = nc.dram_tensor("w", [2 * D, C], F32, kind="ExternalInput").ap()
        out = nc.dram_tensor("out", [HW, HW], F32, kind="ExternalOutput").ap()
        with tile.TileContext(nc) as tc:
            with ExitStack() as ctx:
                _emit(ctx, tc, out, x, w)
        nc.compile()
        _built = nc
    return _built


def kernel(x: np.ndarray, W: np.ndarray) -> np.ndarray:
    nc = _get_nc()
    x = np.asarray(x, dtype=np.float32)
    W = np.ascontiguousarray(np.asarray(W, dtype=np.float32))
    in_maps = [
        {"x": np.ascontiguousarray(x[b].reshape(C, HW)), "w": W} for b in range(B)
    ]
    res = run_bass_kernel_spmd(nc, in_maps, core_ids=list(range(N_CORES)))
    out = np.stack([res.results[b]["out"] for b in range(B)])
    return out[:, None]



# revision 2
# speedup vs baseline: 1.0342x; 1.0342x over previous
"""Spatial self-attention scores kernel for Trainium2 (8 NeuronCores).

Computes, per batch b:
    qk = W @ x_b          # [256, 4096] = [256,256] @ [256,4096]
    q, k = qk[:128], qk[128:]
    sim = (q.T @ k) * 128**-0.5
    out_b = softmax(sim, axis=-1)        # [4096, 4096]
Output: [8, 1, 4096, 4096] float32.

Sharding: data-parallel over batch, one batch image per NeuronCore.

The kernel is HBM-write-bound: 64 MiB of output per core vs ~29 us of
warm compute.  Everything is organized to (a) get the first output DMA
out as early as possible and (b) keep the output queue (sync HWDGE
ring) gapless at the HBM arbitration rate thereafter:

  - x is DMA'd in as fp32 bits re-typed to float32r (bitcast; HWDGE,
    no Q7 descriptor-generation bottleneck), four 1 MiB chunks split
    across BOTH HWDGE rings (scalar + sync) so the two queues drain in
    parallel.  W rides the otherwise-idle gpsimd (SWDGE) queue.
  - The projection runs on the PE straight from the fp32r x (2 cyc/row)
    -- no separate fp16 staging cast.  q/k are evicted from PSUM as
    fp16, so the 4096x4096 attention matmuls stream at 1 cyc/row: even
    a HAM-throttled (cold) PE sustains a group faster than the DMA
    drains it, so the PE can never gate the output queue.
  - Warm-up matmuls bracket the W-transpose so the PE reaches 2.4 GHz
    right as the first x chunk lands.
  - Row-tile 0 runs a fine-grained path: per-1024-column sim pieces and
    exps chase the arriving x chunks, so the first output bytes leave
    ~1.5 us after the last x chunk is projected.
  - per 128-query row-tile: 8 fp16 matmuls (K=128, N=512) into 4-bank
    PSUM tiles; one ScalarE ACTIVATE per 2048 columns computes
    exp(SCALE*sim) with a fused row-sum (accum_out); DVE combines the
    partial sums, takes the reciprocal, and scales the row.
  - output rows leave in 4 MB DMAs (two row-tiles per transfer; the
    first group ships per normalized half-row).
"""

import numpy as np
from contextlib import ExitStack

import concourse.bass as bass
import concourse.tile as tile
from concourse import bacc, mybir
from concourse.bass_utils import run_bass_kernel_spmd
from concourse.masks import make_identity

B = 8
C = 256
HW = 4096
D = 128
SCALE = D ** -0.5
N_CORES = 8

BANK = 512             # PSUM bank width (fp32) = one matmul free-dim
ACT_CHUNK = 2048       # one ScalarE activation spans 4 banks
N_ACT = HW // ACT_CHUNK          # 2
GRP = 2                # row-tiles per output DMA (2 -> 4 MB transfers)
N_GRP = HW // (128 * GRP)        # 16
OUT_BUFS = 4
X_CHUNK = 1024         # x input DMA granularity (4 chunks, 2 per ring)
N_XCHUNK = HW // X_CHUNK         # 4

F32 = mybir.dt.float32
# x lands in SBUF as float32r via a bitcast on the DRAM AP: the HWDGE
# DMA moves the same bits but the tile is *produced* as fp32r, so the
# projection can stream it through the PE at 2 cycles/row.
F32R = mybir.dt.float32r
# q/k live as fp16: the attention matmuls stream at 1 cycle/row and the
# 10-bit mantissa keeps the extra logit noise ~1e-3 (well inside the
# 2e-2 gate; measured end-to-end ~1e-3 scale-relative).
QK_DT = mybir.dt.float16
NWARM_A = 4            # warm-up matmuls before the W transpose
NWARM_B = 4            # ... and after, spanning until x chunk 0 lands


def _emit(ctx: ExitStack, tc: tile.TileContext, out_ap, x_ap, w_ap):
    nc = tc.nc

    const = ctx.enter_context(tc.tile_pool(name="const", bufs=1))
    data = ctx.enter_context(tc.tile_pool(name="data", bufs=1))
    psum = ctx.enter_context(tc.tile_pool(name="psum", bufs=2, space="PSUM"))
    small = ctx.enter_context(tc.tile_pool(name="small", bufs=4))

    warm_f32 = const.tile([128, BANK], F32)
    nc.vector.memset(warm_f32, 0.0)
    warm16 = const.tile([128, BANK], QK_DT)
    nc.vector.tensor_copy(out=warm16, in_=warm_f32)

    # ---- x: four 1 MiB chunks, alternating between the two HWDGE
    # rings (scalar / sync) so they drain in parallel.  fp32r bitcast.
    x_view = x_ap.bitcast(F32R).rearrange("(t p) s -> p t s", p=128)
    x32 = data.tile([128, N_XCHUNK, 2, X_CHUNK], F32R)
    for c in range(N_XCHUNK):
        eng = nc.scalar if c % 2 == 0 else nc.sync
        eng.dma_start(
            out=x32[:, c], in_=x_view[:, :, c * X_CHUNK:(c + 1) * X_CHUNK]
        )

    # ---- W [256, 256] -> SBUF as [p, o_tile, c] on the free SWDGE queue
    w_sb = const.tile([128, 2, C], F32)
    nc.gpsimd.dma_start(out=w_sb, in_=w_ap.rearrange("(t p) c -> p t c", p=128))

    ident = const.tile([128, 128], F32)
    make_identity(nc, ident)

    # pull the exp table load off the first real activation
    tbl = small.tile([128, 1], F32, tag="tbl")
    nc.scalar.activation(
        out=tbl, in_=warm_f32[:, 0:1], func=mybir.ActivationFunctionType.Exp
    )

    # ---- PE warm-up: the HAM clock gate releases to 2.4 GHz only after
    # ~3.4 us of sustained activity; these throwaway matmuls (bracketing
    # the W transpose) make the projection and first sims run warm.
    wps = psum.tile([128, ACT_CHUNK], F32, tag="ps")
    for _ in range(NWARM_A):
        nc.tensor.matmul(
            wps[:, 0:BANK], warm16[:, 0:128], warm16, start=True, stop=True
        )

    # ---- transpose W on PE -> wt_sb[c_sub, c_tile, o] (contraction c on
    # partitions), evicted as fp32r for the 2-cyc/row projection.
    wt_sb = const.tile([128, 2, 2 * D], F32R)
    for t in range(2):          # output-channel tile (q half / k half)
        for ct in range(2):     # input-channel tile
            ps = psum.tile([128, ACT_CHUNK], F32, tag="ps")
            nc.tensor.transpose(
                ps[:, 0:128], w_sb[:, t, ct * 128:(ct + 1) * 128], ident
            )
            nc.vector.tensor_copy(
                out=wt_sb[:, ct, t * 128:(t + 1) * 128], in_=ps[:, 0:128]
            )

    wps2 = psum.tile([128, ACT_CHUNK], F32, tag="ps")
    for _ in range(NWARM_B):
        nc.tensor.matmul(
            wps2[:, 0:BANK], warm16[:, 0:128], warm16, start=True, stop=True
        )

    q_sb = data.tile([128, HW], QK_DT)
    k_sb = data.tile([128, HW], QK_DT)

    def proj_chunk(t, dst, c, banks=(0, 1)):
        """Project output-channel half t (0=q, 1=k) for x chunk c
        (columns [c*X_CHUNK, (c+1)*X_CHUNK)); banks selects the two
        512-wide banks within the chunk."""
        ps = psum.tile([128, ACT_CHUNK], F32, tag="ps")
        for jj in banks:
            lo = jj * BANK
            nc.tensor.matmul(
                ps[:, lo:lo + BANK], wt_sb[:, 0, t * 128:(t + 1) * 128],
                x32[:, c, 0, lo:lo + BANK], start=True, stop=False,
            )
            nc.tensor.matmul(
                ps[:, lo:lo + BANK], wt_sb[:, 1, t * 128:(t + 1) * 128],
                x32[:, c, 1, lo:lo + BANK], start=False, stop=True,
            )
            sl = slice(c * X_CHUNK + lo, c * X_CHUNK + lo + BANK)
            nc.vector.tensor_copy(out=dst[:, sl], in_=ps[:, lo:lo + BANK])

    outp = None
    out_view = out_ap.rearrange("(g t p) m -> g p t m", t=GRP, p=128)

    def sim_chunk(lhs, out_row, a, accum):
        """One 2048-wide slice of one attention row: matmuls + fused exp."""
        ps = psum.tile([128, ACT_CHUNK], F32, tag="ps")
        for jj in range(ACT_CHUNK // BANK):
            sl = slice(a * ACT_CHUNK + jj * BANK, a * ACT_CHUNK + (jj + 1) * BANK)
            nc.tensor.matmul(
                ps[:, jj * BANK:(jj + 1) * BANK], lhs, k_sb[:, sl],
                start=True, stop=True,
            )
        nc.scalar.activation(
            out=out_row[:, a * ACT_CHUNK:(a + 1) * ACT_CHUNK],
            in_=ps[:, 0:ACT_CHUNK],
            func=mybir.ActivationFunctionType.Exp,
            scale=SCALE,
            accum_out=accum,
        )

    def norm_and_ship(out_grp, t, g, sums, split_dma):
        rsum = small.tile([128, 1], F32, tag="rsum")
        nc.vector.tensor_reduce(
            out=rsum, in_=sums, axis=mybir.AxisListType.X,
            op=mybir.AluOpType.add,
        )
        recip = small.tile([128, 1], F32, tag="recip")
        nc.vector.reciprocal(out=recip, in_=rsum)
        if split_dma:
            # normalize and ship each half-row as soon as it is scaled
            # (1 MB transfers) so the first outputs leave immediately
            i = g * GRP + t
            for a in range(N_ACT):
                sl = slice(a * ACT_CHUNK, (a + 1) * ACT_CHUNK)
                nc.vector.tensor_scalar_mul(
                    out=out_grp[:, t, sl], in0=out_grp[:, t, sl],
                    scalar1=recip,
                )
                nc.sync.dma_start(
                    out=out_ap[i * 128:(i + 1) * 128, sl],
                    in_=out_grp[:, t, sl],
                )
        else:
            nc.vector.tensor_scalar_mul(
                out=out_grp[:, t, :], in0=out_grp[:, t, :], scalar1=recip
            )

    def emit_group(g):
        out_grp = outp.tile([128, GRP, HW], F32, tag="out")
        for t in range(GRP):
            lhs = q_sb[:, (g * GRP + t) * 128:(g * GRP + t + 1) * 128]
            sums = small.tile([128, N_ACT], F32, tag="sums")
            for a in range(N_ACT):
                sim_chunk(lhs, out_grp[:, t], a, sums[:, a:a + 1])
            norm_and_ship(out_grp, t, g, sums, False)
        nc.sync.dma_start(out=out_view[g], in_=out_grp)

    # ---- projection + fine-grained first row-tile, chasing the x
    # chunks as they arrive.  k chunk c lands -> project it -> sim piece
    # for row-tile 0 over those 1024 columns -> 1024-wide exp.
    proj_chunk(1, k_sb, 0)              # k cols    0:1024
    proj_chunk(0, q_sb, 0, banks=(0,))  # q rows    0:512 (row-tiles 0-3)

    outp = ctx.enter_context(tc.tile_pool(name="outp", bufs=OUT_BUFS))
    out_g0 = outp.tile([128, GRP, HW], F32, tag="out")
    lhs0 = q_sb[:, 0:128]
    sums0 = small.tile([128, 4], F32, tag="sums")

    def fine_piece(ps, c, acc_i):
        """1024-wide sim piece of row-tile 0 over k cols of chunk c."""
        base = (c % 2) * X_CHUNK
        for jj in range(2):
            sl = slice(c * X_CHUNK + jj * BANK, c * X_CHUNK + (jj + 1) * BANK)
            nc.tensor.matmul(
                ps[:, base + jj * BANK:base + (jj + 1) * BANK], lhs0,
                k_sb[:, sl], start=True, stop=True,
            )
        nc.scalar.activation(
            out=out_g0[:, 0, c * X_CHUNK:(c + 1) * X_CHUNK],
            in_=ps[:, base:base + X_CHUNK],
            func=mybir.ActivationFunctionType.Exp,
            scale=SCALE,
            accum_out=sums0[:, acc_i:acc_i + 1],
        )

    ps01 = psum.tile([128, ACT_CHUNK], F32, tag="ps")
    fine_piece(ps01, 0, 0)
    proj_chunk(1, k_sb, 1)              # k cols 1024:2048
    fine_piece(ps01, 1, 1)
    proj_chunk(1, k_sb, 2)              # k cols 2048:3072
    ps23 = psum.tile([128, ACT_CHUNK], F32, tag="ps")
    fine_piece(ps23, 2, 2)
    proj_chunk(1, k_sb, 3)              # k cols 3072:4096
    fine_piece(ps23, 3, 3)
    norm_and_ship(out_g0, 0, 0, sums0, True)

    # row-tile 1 completes group 0 (also shipped as scaled half-rows)
    lhs = q_sb[:, 128:256]
    sums = small.tile([128, N_ACT], F32, tag="sums")
    for a in range(N_ACT):
        sim_chunk(lhs, out_g0[:, 1], a, sums[:, a:a + 1])
    norm_and_ship(out_g0, 1, 0, sums, True)

    # remaining q projections trickle in one 512-wide bank at a time,
    # each just ahead of the first group that reads it
    emit_group(1)
    for g in range(2, N_GRP):
        if g % 2 == 0:
            b = g // 2
            proj_chunk(0, q_sb, b // 2, banks=(b % 2,))
        emit_group(g)


_built = None


def _get_nc():
    global _built
    if _built is None:
        nc = bacc.Bacc("TRN2", target_bir_lowering=False, debug=False)
        x = nc.dram_tensor("x", [C, HW], F32, kind="ExternalInput").ap()
        w = nc.dram_tensor("w", [2 * D, C], F32, kind="ExternalInput").ap()
        out = nc.dram_tensor("out", [HW, HW], F32, kind="ExternalOutput").ap()
        with tile.TileContext(nc) as tc:
            with ExitStack() as ctx:
                _emit(ctx, tc, out, x, w)
        nc.compile()
        _built = nc
    return _built


def kernel(x: np.ndarray, W: np.ndarray) -> np.ndarray:
    nc = _get_nc()
    x = np.asarray(x, dtype=np.float32)
    W = np.ascontiguousarray(np.asarray(W, dtype=np.float32))
    in_maps = [
        {"x": np.ascontiguousarray(x[b].reshape(C, HW)), "w": W} for b in range(B)
    ]
    res = run_bass_kernel_spmd(nc, in_maps, core_ids=list(range(N_CORES)))
    out = np.stack([res.results[b]["out"] for b in range(B)])
    return out[:, None]


# revision 4
# speedup vs baseline: 1.0640x; 1.0289x over previous
"""Spatial self-attention scores kernel for Trainium2 (8 NeuronCores).

Computes, per batch b:
    qk = W @ x_b          # [256, 4096] = [256,256] @ [256,4096]
    q, k = qk[:128], qk[128:]
    sim = (q.T @ k) * 128**-0.5
    out_b = softmax(sim, axis=-1)        # [4096, 4096]
Output: [8, 1, 4096, 4096] float32.

Sharding: data-parallel over batch, one batch image per NeuronCore.

The kernel is HBM-write-bound: 64 MiB of output per core vs ~29 us of
warm compute.  Everything is organized to (a) get the first output DMA
out as early as possible and (b) keep the output queue (sync HWDGE
ring) gapless at the HBM arbitration rate thereafter:

  - x is DMA'd in as fp32 bits re-typed to float32r (bitcast; HWDGE,
    no Q7 descriptor-generation bottleneck), four 1 MiB chunks split
    across BOTH HWDGE rings (scalar + sync) so the two queues drain in
    parallel.  W rides the otherwise-idle gpsimd (SWDGE) queue.
  - The projection runs on the PE straight from the fp32r x (2 cyc/row)
    -- no separate fp16 staging cast.  q/k are evicted from PSUM as
    fp16, so the 4096x4096 attention matmuls stream at 1 cyc/row: even
    a HAM-throttled (cold) PE sustains a group faster than the DMA
    drains it, so the PE can never gate the output queue.
  - Warm-up matmuls bracket the W-transpose so the PE reaches 2.4 GHz
    right as the first x chunk lands.
  - Row-tile 0 runs a fine-grained path: per-1024-column sim pieces and
    exps chase the arriving x chunks, so the first output bytes leave
    ~1.5 us after the last x chunk is projected.
  - per 128-query row-tile: 8 fp16 matmuls (K=128, N=512) into 4-bank
    PSUM tiles; one ScalarE ACTIVATE per 2048 columns computes
    exp(SCALE*sim) with a fused row-sum (accum_out); DVE combines the
    partial sums, takes the reciprocal, and scales the row.
  - output rows leave in 4 MB DMAs (two row-tiles per transfer; the
    first group ships per normalized half-row).
"""

import numpy as np
from contextlib import ExitStack

import concourse.bass as bass
import concourse.tile as tile
from concourse import bacc, mybir
from concourse.bass_utils import run_bass_kernel_spmd
from concourse.masks import make_identity

B = 8
C = 256
HW = 4096
D = 128
SCALE = D ** -0.5
N_CORES = 8

BANK = 512             # PSUM bank width (fp32) = one matmul free-dim
ACT_CHUNK = 2048       # one ScalarE activation spans 4 banks
N_ACT = HW // ACT_CHUNK          # 2
GRP = 2                # row-tiles per output DMA (2 -> 4 MB transfers)
N_GRP = HW // (128 * GRP)        # 16
OUT_BUFS = 4
X_CHUNK = 1024         # x input DMA granularity (4 chunks, 2 per ring)
N_XCHUNK = HW // X_CHUNK         # 4

F32 = mybir.dt.float32
# x lands in SBUF as float32r via a bitcast on the DRAM AP: the HWDGE
# DMA moves the same bits but the tile is *produced* as fp32r, so the
# projection can stream it through the PE at 2 cycles/row.
F32R = mybir.dt.float32r
# q/k live as fp16: the attention matmuls stream at 1 cycle/row and the
# 10-bit mantissa keeps the extra logit noise ~1e-3 (well inside the
# 2e-2 gate; measured end-to-end ~1e-3 scale-relative).
QK_DT = mybir.dt.float16
NWARM_A = 2            # warm-up matmuls before the W transpose
NWARM_B = 2            # ... and after, spanning until x chunk 0 lands


def _emit(ctx: ExitStack, tc: tile.TileContext, out_ap, x_ap, w_ap):
    nc = tc.nc

    const = ctx.enter_context(tc.tile_pool(name="const", bufs=1))
    data = ctx.enter_context(tc.tile_pool(name="data", bufs=1))
    psum = ctx.enter_context(tc.tile_pool(name="psum", bufs=2, space="PSUM"))
    small = ctx.enter_context(tc.tile_pool(name="small", bufs=4))

    warm_f32 = const.tile([128, BANK], F32)
    nc.vector.memset(warm_f32, 0.0)
    warm16 = const.tile([128, BANK], QK_DT)
    nc.vector.tensor_copy(out=warm16, in_=warm_f32)

    # ---- W [256, 256] first, on the scalar HWDGE ring: it gates the
    # transpose (and with it the whole projection pipeline), and at
    # 256 KB costs the ring less than 1 us.
    w_sb = const.tile([128, 2, C], F32)
    nc.scalar.dma_start(out=w_sb, in_=w_ap.rearrange("(t p) c -> p t c", p=128))

    # ---- x: four 1 MiB chunks, alternating between the two HWDGE
    # rings so they drain in parallel; chunk 0 (which unblocks the
    # first projection) rides the W-free sync ring.  fp32r bitcast.
    x_view = x_ap.bitcast(F32R).rearrange("(t p) s -> p t s", p=128)
    x32 = data.tile([128, N_XCHUNK, 2, X_CHUNK], F32R)
    for c in range(N_XCHUNK):
        eng = nc.sync if c % 2 == 0 else nc.scalar
        eng.dma_start(
            out=x32[:, c], in_=x_view[:, :, c * X_CHUNK:(c + 1) * X_CHUNK]
        )

    ident = const.tile([128, 128], F32)
    make_identity(nc, ident)

    # pull the exp table load off the first real activation
    tbl = small.tile([128, 1], F32, tag="tbl")
    nc.scalar.activation(
        out=tbl, in_=warm_f32[:, 0:1], func=mybir.ActivationFunctionType.Exp
    )

    # ---- PE warm-up: the HAM clock gate releases to 2.4 GHz only after
    # ~3.4 us of sustained activity; these throwaway matmuls (bracketing
    # the W transpose) make the projection and first sims run warm.
    wps = psum.tile([128, ACT_CHUNK], F32, tag="ps")
    for _ in range(NWARM_A):
        nc.tensor.matmul(
            wps[:, 0:BANK], warm16[:, 0:128], warm16, start=True, stop=True
        )

    # ---- transpose W on PE -> wt_sb[c_sub, c_tile, o] (contraction c on
    # partitions), evicted as fp32r for the 2-cyc/row projection.
    wt_sb = const.tile([128, 2, 2 * D], F32R)
    for t in range(2):          # output-channel tile (q half / k half)
        for ct in range(2):     # input-channel tile
            ps = psum.tile([128, ACT_CHUNK], F32, tag="ps")
            nc.tensor.transpose(
                ps[:, 0:128], w_sb[:, t, ct * 128:(ct + 1) * 128], ident
            )
            nc.vector.tensor_copy(
                out=wt_sb[:, ct, t * 128:(t + 1) * 128], in_=ps[:, 0:128]
            )

    wps2 = psum.tile([128, ACT_CHUNK], F32, tag="ps")
    for _ in range(NWARM_B):
        nc.tensor.matmul(
            wps2[:, 0:BANK], warm16[:, 0:128], warm16, start=True, stop=True
        )

    q_sb = data.tile([128, HW], QK_DT)
    k_sb = data.tile([128, HW], QK_DT)

    def proj_chunk(t, dst, c, banks=(0, 1)):
        """Project output-channel half t (0=q, 1=k) for x chunk c
        (columns [c*X_CHUNK, (c+1)*X_CHUNK)); banks selects the two
        512-wide banks within the chunk."""
        ps = psum.tile([128, ACT_CHUNK], F32, tag="ps")
        for jj in banks:
            lo = jj * BANK
            nc.tensor.matmul(
                ps[:, lo:lo + BANK], wt_sb[:, 0, t * 128:(t + 1) * 128],
                x32[:, c, 0, lo:lo + BANK], start=True, stop=False,
            )
            nc.tensor.matmul(
                ps[:, lo:lo + BANK], wt_sb[:, 1, t * 128:(t + 1) * 128],
                x32[:, c, 1, lo:lo + BANK], start=False, stop=True,
            )
            sl = slice(c * X_CHUNK + lo, c * X_CHUNK + lo + BANK)
            nc.vector.tensor_copy(out=dst[:, sl], in_=ps[:, lo:lo + BANK])

    outp = None
    out_view = out_ap.rearrange("(g t p) m -> g p t m", t=GRP, p=128)

    def sim_chunk(lhs, out_row, a, accum):
        """One 2048-wide slice of one attention row: matmuls + fused exp."""
        ps = psum.tile([128, ACT_CHUNK], F32, tag="ps")
        for jj in range(ACT_CHUNK // BANK):
            sl = slice(a * ACT_CHUNK + jj * BANK, a * ACT_CHUNK + (jj + 1) * BANK)
            nc.tensor.matmul(
                ps[:, jj * BANK:(jj + 1) * BANK], lhs, k_sb[:, sl],
                start=True, stop=True,
            )
        nc.scalar.activation(
            out=out_row[:, a * ACT_CHUNK:(a + 1) * ACT_CHUNK],
            in_=ps[:, 0:ACT_CHUNK],
            func=mybir.ActivationFunctionType.Exp,
            scale=SCALE,
            accum_out=accum,
        )

    def norm_and_ship(out_grp, t, g, sums, split_dma):
        rsum = small.tile([128, 1], F32, tag="rsum")
        nc.vector.tensor_reduce(
            out=rsum, in_=sums, axis=mybir.AxisListType.X,
            op=mybir.AluOpType.add,
        )
        recip = small.tile([128, 1], F32, tag="recip")
        nc.vector.reciprocal(out=recip, in_=rsum)
        if split_dma:
            # normalize and ship each half-row as soon as it is scaled
            # (1 MB transfers) so the first outputs leave immediately
            i = g * GRP + t
            for a in range(N_ACT):
                sl = slice(a * ACT_CHUNK, (a + 1) * ACT_CHUNK)
                nc.vector.tensor_scalar_mul(
                    out=out_grp[:, t, sl], in0=out_grp[:, t, sl],
                    scalar1=recip,
                )
                nc.sync.dma_start(
                    out=out_ap[i * 128:(i + 1) * 128, sl],
                    in_=out_grp[:, t, sl],
                )
        else:
            nc.vector.tensor_scalar_mul(
                out=out_grp[:, t, :], in0=out_grp[:, t, :], scalar1=recip
            )

    def emit_group(g):
        out_grp = outp.tile([128, GRP, HW], F32, tag="out")
        for t in range(GRP):
            lhs = q_sb[:, (g * GRP + t) * 128:(g * GRP + t + 1) * 128]
            sums = small.tile([128, N_ACT], F32, tag="sums")
            for a in range(N_ACT):
                sim_chunk(lhs, out_grp[:, t], a, sums[:, a:a + 1])
            norm_and_ship(out_grp, t, g, sums, False)
        nc.sync.dma_start(out=out_view[g], in_=out_grp)

    # ---- projection + fine-grained first row-tile, chasing the x
    # chunks as they arrive.  k chunk c lands -> project it -> sim piece
    # for row-tile 0 over those 1024 columns -> 1024-wide exp.
    proj_chunk(1, k_sb, 0)              # k cols    0:1024
    proj_chunk(0, q_sb, 0, banks=(0,))  # q rows    0:512 (row-tiles 0-3)

    outp = ctx.enter_context(tc.tile_pool(name="outp", bufs=OUT_BUFS))
    out_g0 = outp.tile([128, GRP, HW], F32, tag="out")
    lhs0 = q_sb[:, 0:128]
    sums0 = small.tile([128, 4], F32, tag="sums")

    def fine_piece(ps, c, acc_i):
        """1024-wide sim piece of row-tile 0 over k cols of chunk c."""
        base = (c % 2) * X_CHUNK
        for jj in range(2):
            sl = slice(c * X_CHUNK + jj * BANK, c * X_CHUNK + (jj + 1) * BANK)
            nc.tensor.matmul(
                ps[:, base + jj * BANK:base + (jj + 1) * BANK], lhs0,
                k_sb[:, sl], start=True, stop=True,
            )
        nc.scalar.activation(
            out=out_g0[:, 0, c * X_CHUNK:(c + 1) * X_CHUNK],
            in_=ps[:, base:base + X_CHUNK],
            func=mybir.ActivationFunctionType.Exp,
            scale=SCALE,
            accum_out=sums0[:, acc_i:acc_i + 1],
        )

    ps01 = psum.tile([128, ACT_CHUNK], F32, tag="ps")
    fine_piece(ps01, 0, 0)
    proj_chunk(1, k_sb, 1)              # k cols 1024:2048
    fine_piece(ps01, 1, 1)
    proj_chunk(1, k_sb, 2)              # k cols 2048:3072
    ps23 = psum.tile([128, ACT_CHUNK], F32, tag="ps")
    fine_piece(ps23, 2, 2)
    proj_chunk(1, k_sb, 3)              # k cols 3072:4096
    fine_piece(ps23, 3, 3)
    norm_and_ship(out_g0, 0, 0, sums0, True)

    # row-tile 1 completes group 0 (also shipped as scaled half-rows)
    lhs = q_sb[:, 128:256]
    sums = small.tile([128, N_ACT], F32, tag="sums")
    for a in range(N_ACT):
        sim_chunk(lhs, out_g0[:, 1], a, sums[:, a:a + 1])
    norm_and_ship(out_g0, 1, 0, sums, True)

    # remaining q projections trickle in one 512-wide bank at a time,
    # each just ahead of the first group that reads it
    emit_group(1)
    for g in range(2, N_GRP):
        if g % 2 == 0:
            b = g // 2
            proj_chunk(0, q_sb, b // 2, banks=(b % 2,))
        emit_group(g)


_built = None


def _get_nc():
    global _built
    if _built is None:
        nc = bacc.Bacc("TRN2", target_bir_lowering=False, debug=False)
        x = nc.dram_tensor("x", [C, HW], F32, kind="ExternalInput").ap()
        w = nc.dram_tensor("w", [2 * D, C], F32, kind="ExternalInput").ap()
        out = nc.dram_tensor("out", [HW, HW], F32, kind="ExternalOutput").ap()
        with tile.TileContext(nc) as tc:
            with ExitStack() as ctx:
                _emit(ctx, tc, out, x, w)
        nc.compile()
        _built = nc
    return _built


def kernel(x: np.ndarray, W: np.ndarray) -> np.ndarray:
    nc = _get_nc()
    x = np.asarray(x, dtype=np.float32)
    W = np.ascontiguousarray(np.asarray(W, dtype=np.float32))
    in_maps = [
        {"x": np.ascontiguousarray(x[b].reshape(C, HW)), "w": W} for b in range(B)
    ]
    res = run_bass_kernel_spmd(nc, in_maps, core_ids=list(range(N_CORES)))
    out = np.stack([res.results[b]["out"] for b in range(B)])
    return out[:, None]
